# revision 27
# baseline (speedup 1.0000x reference)
"""GAT layer (message passing) on 8 Trainium2 NeuronCores via Bass/Tile.

Strategy (src-sharded, dst-sectioned, K-padded node-aligned slots):
  - 8 cores, each owns nodes [c*NPC, (c+1)*NPC) (src sharding -> segment
    sums stay core-local).
  - Host passes x^T ROTATED per core so the core's own shard maps to table
    rows [0, NPC): all program constants become core-independent; per-core
    differences live only in index input data.
  - Phase 1 (per core, replicated work): compute per-node row
    G[n] = [h(64xbf16) | t(f32) | pad] via TensorE from xT chunks,
    where h = x@W, t = h@a2.  G lives in DRAM as one tile PER SECTION
    (section sizes [2560, 32256, 32256, 32256, 672]: int16 dma_gather
    indices stay in range, a small first section shortens the phase-1
    prologue, and big middle sections minimize per-(node,section)
    scatter rows).  Phase 1b (s = h@a1 in slot-node order, from
    host-gathered xS) is interleaved per section, one section behind
    phase 1, so early batches unblock asap.
  - Self-loops are NOT materialized as gather slots; their contribution
    (exp(lrelu(s_n+t_n)) and *h_n) is added in the final pass from
    dense reads of the core's own G rows plus an s_own table.
  - Phase 2, software-pipelined with a 2-batch emission skew: per batch,
    dma_gather slot rows (256B) by dst (SWDGE queue 0), compute
    w = exp(leaky_relu(s+t)) and prod = w*h on ACT/DVE, reduce each
    node's K slots to f32 (DVE), cast to a bf16 row [h'(64)|e_sum|pad]
    (Scalar engine) and dma_scatter_add (256B rows, SWDGE queue 1) into
    one of TWO bf16 DRAM accumulators (even/odd batches).  Within one
    accumulator, scatters are serialized by partial-buffer reuse; across
    the two, rows never collide -> no RMW races, transfers overlap.
    Pool-engine SWDGE descriptor generation (~7.5ns/descriptor) is the
    kernel's critical path; everything else overlaps under it.
  - Final: out = (h'0 + h'1 + w_self*h_n) / (e0 + e1 + w_self) in f32.
"""

import numpy as np

N = 100000
E = 1600000
IN_C = 128
OUT_C = 64
ALPHA = 0.2
NCORES = 8
EL = 128                       # bf16 elements per table row (256B)
GB = 4                         # phase-1 chunks (of 128 nodes) per group
MAXC = 64                      # max slot-columns per batch (nidx<=8192)
MAXM = 64                      # max node-columns per batch
SEC_SIZES = (10240, 29952, 29952, 29856)
M_COST = 1.3   # relative cost of one node-column vs one slot-column


def _configure():
    """Set the graph-size-derived constants (module globals)."""
    global NPC, NPC_PAD, ACC_TRASH, ACC_ROWS, NSEC, SEC_OFF, SEC_PAD, XP
    NPC = N // NCORES
    NPC_PAD = ((NPC + 127) // 128) * 128
    ACC_TRASH = NPC_PAD
    ACC_ROWS = NPC_PAD + 128
    assert sum(SEC_SIZES) == N
    NSEC = len(SEC_SIZES)
    SEC_OFF = np.concatenate([[0], np.cumsum(SEC_SIZES)])  # [NSEC+1]
    # padded (512-multiple) row count per section; trash row sits at
    # SEC_PAD[s] which must stay <= 32767 for int16 gather indices
    SEC_PAD = [((sz + 511) // 512) * 512 for sz in SEC_SIZES]
    for sp_ in SEC_PAD:
        assert sp_ <= 32767
    XP = sum(SEC_PAD)
    assert XP % (128 * GB) == 0


_configure()


def _wrap16(vals):
    """[128, cols] rank layout (rank i -> (p=i%128, col=i//128)) ->
    dma_gather idx tensor [128, (128*cols)/16] int16 (wrapped, replicated)."""
    L = vals.T.reshape(-1)  # rank order
    n = L.shape[0]
    w = L.reshape(n // 16, 16).T  # [16, n/16]
    return np.tile(w, (8, 1)).astype(np.int16)


def _blocked_map(total):
    """Permutation old-row -> blocked row for a table written in 1024-row
    blocks (last block may be 512 or 256): within a block, row j*128+p is
    stored at p*jd+j (jd = block_rows/128) so each partition's jd rows are
    contiguous in DRAM (2KB pieces instead of 256B sprays)."""
    r = np.arange(total, dtype=np.int64)
    b = r // 1024
    off = r - b * 1024
    bs = np.minimum(1024, total - b * 1024)
    jd = bs // 128
    j, p = off // 128, off % 128
    return b * 1024 + p * jd + j


_G_MAPS = None


def _g_maps():
    global _G_MAPS
    if _G_MAPS is None:
        _G_MAPS = [np.concatenate([_blocked_map(SEC_PAD[s_]), [SEC_PAD[s_]]])
                   for s_ in range(NSEC)] + [
                   np.concatenate([_blocked_map(NPC_PAD), [ACC_TRASH]])]
    return _G_MAPS


def _pick_buckets(deg_cc):
    """DP bucket selection for one section.

    deg_cc: [NCORES, NPC] per-node degree (0 = absent; absent nodes take no
    slot).  Returns ascending bucket list kb_s minimizing
    sum_b cols_b * (K_b + M_COST) where cols_b = max over cores of
    ceil(#nodes with degree in (prev, K_b] / 128)."""
    maxd = int(deg_cc.max())
    if maxd == 0:
        return [1]
    # cum[c][d] = #nodes of core c with 1 <= degree <= d
    cum = np.zeros((NCORES, maxd + 1), np.int64)
    for c in range(NCORES):
        h = np.bincount(deg_cc[c][deg_cc[c] > 0], minlength=maxd + 1)
        cum[c] = np.cumsum(h)
    INF = float("inf")
    best = [0.0] + [INF] * maxd
    prev = [0] * (maxd + 1)
    for d in range(1, maxd + 1):
        for d0 in range(d):
            if best[d0] == INF:
                continue
            cols = int(np.max(-(-(cum[:, d] - cum[:, d0]) // 128)))
            cost = best[d0] + cols * (d + M_COST)
            if cost < best[d]:
                best[d] = cost
                prev[d] = d0
    kb_s = []
    d = maxd
    while d > 0:
        kb_s.append(d)
        d = prev[d]
    return sorted(kb_s)


def _prep(edge_index):
    """Pure-integer host prep: per-core rotated sections, classes, batches.

    Returns (meta, per_core) where meta is core-independent (defines the
    program) and per_core holds the index input tensors."""
    src = np.asarray(edge_index[0], dtype=np.int64)
    dst = np.asarray(edge_index[1], dtype=np.int64)

    core_of = src // NPC

    # ---- pass 1: per-core degree tables in rotated-section space ----
    per_core_raw = []
    cnt_all = np.zeros((NCORES, NPC, NSEC), np.int64)
    for c in range(NCORES):
        m = core_of == c
        sl = (src[m] - c * NPC).astype(np.int64)
        rot = (dst[m] - c * NPC) % N  # rotated dst position
        sec = np.searchsorted(SEC_OFF, rot, side="right") - 1
        row = (rot - SEC_OFF[sec]).astype(np.int32)
        key = sl * NSEC + sec
        order = np.argsort(key, kind="stable")
        rows_sorted = row[order]
        cnt = np.bincount(key, minlength=NPC * NSEC).reshape(NPC, NSEC)
        offs = np.zeros(NPC * NSEC + 1, np.int64)
        np.cumsum(cnt.ravel(), out=offs[1:])
        cnt_all[c] = cnt
        per_core_raw.append((cnt, rows_sorted, offs))

    # sorted-degree capacity profiles: per section, sort each core's
    # (deg>0) nodes by degree descending; column i's capacity is the max
    # over cores of the sorted profile at rank 128*i.  Every core's rank-r
    # node fits its column by majorization, so there is no class padding.
    zd_any = cnt_all.sum(axis=2) == 0  # [NCORES, NPC]
    sec_runs = []   # per section: list of (K, ncols) runs, K descending
    sec_caps = []   # per section: per-column capacity array
    for s_ in range(NSEC):
        deg = cnt_all[:, :, s_].copy()
        if s_ == 0:
            deg[zd_any] = 1
        prof = -np.sort(-deg, axis=1)  # [NCORES, NPC] descending
        cap = prof.max(axis=0)
        ncols = int(np.sum(cap[::128] > 0))
        cap_col = cap[::128][:ncols].astype(np.int64)
        runs = []
        i = 0
        while i < ncols:
            j = i
            while j < ncols and cap_col[j] == cap_col[i]:
                j += 1
            runs.append((int(cap_col[i]), j - i))
            i = j
        sec_runs.append(runs)
        sec_caps.append(cap_col)

    # per-core node order (degree descending, stable by node id)
    for c in range(NCORES):
        cnt, rows_sorted, offs = per_core_raw[c]
        orders = []
        for s_ in range(NSEC):
            deg = cnt[:, s_].copy()
            if s_ == 0:
                deg[zd_any[c]] = 1
            sel = np.where(deg > 0)[0]
            order = sel[np.argsort(-deg[sel], kind="stable")]
            orders.append(order)
        per_core_raw[c] = (cnt, orders, rows_sorted, offs)

    # ---- shared metadata: batch structure from capacity runs ----
    total_slots = int(sum(sum(k * n for k, n in sec_runs[s_])
                          for s_ in range(NSEC)) * 128)
    # batches: list of dicts(sec, runs=[(K, m_run)], cols, m)
    batches = []
    for s_ in range(NSEC):
        cur = {"sec": s_, "runs": [], "cols": 0, "m": 0}
        for K, m_all in sec_runs[s_]:
            m_left = m_all
            while m_left > 0:
                mfit_cols = (MAXC - cur["cols"]) // K
                mfit = min(m_left, mfit_cols, MAXM - cur["m"])
                if mfit <= 0:
                    if cur["runs"]:
                        batches.append(cur)
                    cur = {"sec": s_, "runs": [], "cols": 0, "m": 0}
                    continue
                cur["runs"].append((K, mfit))
                cur["cols"] += mfit * K
                cur["m"] += mfit
                m_left -= mfit
        if cur["runs"]:
            batches.append(cur)
    # rank base per batch in sorted order, then interleave big-K/small-K
    # batches so large scatter batches sit next to large gather batches
    # (scatter desc-gen hides under gather desc-gen on the other queue)
    inter = []
    for s_ in range(NSEC):
        bs = [b for b in batches if b["sec"] == s_]
        base = 0
        for b in bs:
            b["rank0"] = base
            base += b["m"] * 128
        lo, hi = 0, len(bs) - 1
        left = True
        while lo <= hi:
            if left:
                inter.append(bs[lo])
                lo += 1
            else:
                inter.append(bs[hi])
                hi -= 1
            left = not left
    batches = inter
    meta = {"batches": batches,
            "gcols": sum(b["cols"] for b in batches),
            "mcols": sum(b["m"] for b in batches),
            "inflation": total_slots / max(E, 1) * NCORES}

    # ---- pass 2: fill per-core index tensors ----
    per_core = []
    for c in range(NCORES):
        cnt, orders, rows_sorted, offs = per_core_raw[c]
        for b in batches:
            s_ = b["sec"]
            gmat = np.full((128, b["cols"]), SEC_PAD[s_], np.int64)  # TRASH
            smat = np.full((128, b["m"]), ACC_TRASH, np.int64)
            b["_fill"] = (gmat, smat)
        for b in batches:
            s_ = b["sec"]
            order = orders[s_]
            gmat, smat = b["_fill"]
            co, mo = 0, 0
            rank_local = 0
            for K, m_run in b["runs"]:
                r0 = b["rank0"] + rank_local
                av_all = order[r0:r0 + m_run * 128]
                nn = len(av_all)
                if nn:
                    rr = np.arange(nn)
                    pp = rr % 128
                    cc = rr // 128
                    smat[pp, mo + cc] = av_all
                    d = cnt[av_all, s_]
                    base = offs[av_all * NSEC + s_]
                    idxmat = base[:, None] + np.arange(K)[None, :]
                    valid = np.arange(K)[None, :] < d[:, None]
                    vals = np.where(
                        valid,
                        rows_sorted[np.minimum(idxmat,
                                               max(len(rows_sorted), 1)
                                               - 1)],
                        SEC_PAD[s_])
                    vals = _g_maps()[s_][vals]
                    # slot columns for node at (p, col): co + cc*K .. +K
                    colidx = (mo * 0 + co) + cc[:, None] * K + \
                        np.arange(K)[None, :]
                    gmat[pp[:, None].repeat(K, 1), colidx] = vals
                rank_local += m_run * 128
                co += m_run * K
                mo += m_run
        # wrap all batches
        gidx_blocks = []
        sidx_blocks = []
        sorder_blocks = []
        accmap = _g_maps()[NSEC]
        for b in batches:
            gmat, smat = b["_fill"]
            gidx_blocks.append(_wrap16(gmat))
            sidx_blocks.append(_wrap16(accmap[smat]))
            sorder_blocks.append(smat)
        per_core.append({
            "gidx": np.concatenate(gidx_blocks, axis=1),
            "sidx": np.concatenate(sidx_blocks, axis=1),
            "sorder": np.concatenate(sorder_blocks, axis=1),
        })
        for b in batches:
            del b["_fill"]
    return meta, per_core


def _build_program(meta):
    import concourse.bacc as bacc
    import concourse.tile as tile
    from concourse import mybir
    from concourse.masks import make_identity

    f32 = mybir.dt.float32
    bf16 = mybir.dt.bfloat16
    i16 = mybir.dt.int16
    AF = mybir.ActivationFunctionType
    OP = mybir.AluOpType
    AX = mybir.AxisListType
    batches = meta["batches"]
    gcols_total = meta["gcols"]
    mcols_total = meta["mcols"]
    nbat = len(batches)
    # prefix offsets per batch (wrapped-idx cols and node cols)
    goffs = np.concatenate([[0], np.cumsum([b["cols"] * 8 for b in batches])])
    moffs = np.concatenate([[0], np.cumsum([b["m"] * 8 for b in batches])])

    nc = bacc.Bacc(None, target_bir_lowering=False, num_swdge_queues=2)
    xT = nc.dram_tensor("xT", [128, XP], bf16, kind="ExternalInput")
    Wd = nc.dram_tensor("W", [IN_C, OUT_C], f32, kind="ExternalInput")
    aT = nc.dram_tensor("aT", [OUT_C, 2], f32, kind="ExternalInput")
    gidx = nc.dram_tensor("gidx", [128, gcols_total * 8], i16,
                          kind="ExternalInput")
    sidx = nc.dram_tensor("sidx", [128, mcols_total * 8], i16,
                          kind="ExternalInput")
    mcols_pad = -(-mcols_total // GB) * GB
    xS = nc.dram_tensor("xS", [128, mcols_pad * 128], bf16,
                        kind="ExternalInput")
    # bf16 accumulator rows: [h'(64) | e_sum | pad] = 128 bf16 = 256B.
    # Two accumulators (even/odd batches): concurrent in-flight scatter_adds
    # never touch the same tensor, and same-parity scatters are serialized
    # by partial-buffer reuse (bufs=2) -> no RMW races.
    acc0 = nc.dram_tensor("acc0", [ACC_ROWS, EL], bf16, kind="ExternalOutput")
    acc1 = nc.dram_tensor("acc1", [ACC_ROWS, EL], bf16, kind="ExternalOutput")
    acc2 = nc.dram_tensor("acc2", [NPC_PAD, EL], bf16, kind="ExternalOutput")
    out = nc.dram_tensor("out", [NPC_PAD, OUT_C], f32, kind="ExternalOutput")

    with tile.TileContext(nc) as tc:
        with (
            tc.tile_pool(name="dram", bufs=1, space="DRAM") as dpool,
            tc.tile_pool(name="setup", bufs=1) as setup,
            tc.tile_pool(name="xin", bufs=2) as xin,
            tc.tile_pool(name="ps", bufs=2, space="PSUM") as psp,
            tc.tile_pool(name="gout", bufs=2) as gop,
            tc.tile_pool(name="ph2", bufs=3) as ph2,
            tc.tile_pool(name="idx", bufs=5) as idxp,
            tc.tile_pool(name="hgp", bufs=4) as hgp,
            tc.tile_pool(name="ph2b", bufs=2) as ph2b,
            tc.tile_pool(name="pfp", bufs=1) as pfp,
            tc.tile_pool(name="fin", bufs=2) as fin,
            tc.tile_pool(name="fin2", bufs=2) as fin2,
        ):
            Gs = []
            for s_ in range(NSEC):
                gsec = dpool.tile([SEC_PAD[s_] + 128, EL], bf16,
                                  tag=f"gsec{s_}", name=f"gsec{s_}")
                Gs.append(gsec)

            # ---------- setup: W, Wa1, Wa2 ----------
            ident = setup.tile([128, 128], f32)
            make_identity(nc, ident[:])
            Wt = setup.tile([128, OUT_C], f32)
            nc.sync.dma_start(Wt[:], Wd[:])
            aTt = setup.tile([OUT_C, 2], f32)
            nc.sync.dma_start(aTt[:], aT[:])
            WT_ps = psp.tile([OUT_C, 128], f32, tag="wt")
            nc.tensor.transpose(out=WT_ps[:], in_=Wt[:], identity=ident[:])
            WT = setup.tile([OUT_C, 128], f32)
            nc.vector.tensor_copy(WT[:], WT_ps[:])
            Wa_ps = psp.tile([128, 2], f32, tag="wa")
            nc.tensor.matmul(Wa_ps[:], WT[:], aTt[:], start=True, stop=True)
            # rhs = [W | Wa2] (t rides in G); Wa1 separate for phase 1b (s)
            rhs = setup.tile([128, OUT_C + 1], bf16)
            nc.vector.tensor_copy(rhs[:, 0:OUT_C], Wt[:])
            nc.vector.tensor_copy(rhs[:, OUT_C:OUT_C + 1], Wa_ps[:, 1:2])
            wa1 = setup.tile([128, 1], bf16)
            nc.vector.tensor_copy(wa1[:], Wa_ps[:, 0:1])
            wa12 = setup.tile([128, 2], bf16)
            nc.vector.tensor_copy(wa12[:], Wa_ps[:, 0:2])
            # per-section s tiles so early batches don't wait on all of 1b
            sec_start, secs = {}, []
            for k, b in enumerate(batches):
                if b["sec"] not in sec_start:
                    sec_start[b["sec"]] = moffs[k] // 8
                    secs.append(b["sec"])
            sec_end = {
                s_: (sec_start[secs[i + 1]] if i + 1 < len(secs)
                     else mcols_total)
                for i, s_ in enumerate(secs)
            }
            s_tiles = {}
            for s_ in secs:
                stile = setup.tile(
                    [128, sec_end[s_] - sec_start[s_]], f32,
                    tag=f"sres{s_}", name=f"sres{s_}")
                s_tiles[s_] = stile
            # s,t of own nodes, final-pass (acc-row) order
            nout_cols = NPC_PAD // 128
            s_own = setup.tile([128, nout_cols], f32, tag="sown")
            t_own = setup.tile([128, nout_cols], f32, tag="town")

            # special TRASH rows: h=0, t=-1e30, s=0
            sp = setup.tile([1, EL], bf16)
            nc.vector.memset(sp[:], 0)
            spf = sp[:].bitcast(f32)  # [1, 64]
            nc.vector.memset(spf[0:1, 32:33], -1.0e30)
            for s_ in range(NSEC):
                nc.sync.dma_start(Gs[s_][SEC_PAD[s_]:SEC_PAD[s_] + 1, :],
                                  sp[:])

            # ---------- phase 1 + 1b ----------
            # global 512-row group index ranges per section
            sec_g0 = np.concatenate(
                [[0], np.cumsum([sp_ // 512 for sp_ in SEC_PAD])])
            n_sgroups = mcols_pad // GB
            XTILE = 4096
            groups_per_xtile = max(1, XTILE // (128 * GB))

            def emit_p1_section(s_):
                glo, ghi = int(sec_g0[s_]), int(sec_g0[s_ + 1])
                xt_t = None
                gb_t = None
                for g in range(glo, ghi):
                    if (g - glo) % groups_per_xtile == 0:
                        xt_t = xin.tile([128, XTILE], bf16, tag="xt")
                        x0 = g * 128 * GB
                        xlen = min(XTILE, XP - x0)
                        nc.sync.dma_start(xt_t[:, 0:xlen],
                                          xT[:, x0:x0 + xlen])
                    ps = psp.tile([128, GB, OUT_C + 1], f32, tag="hps")
                    for j in range(GB):
                        off = ((g - glo) % groups_per_xtile) * 128 * GB + \
                            j * 128
                        nc.tensor.matmul(ps[:, j, :], xt_t[:, off:off + 128],
                                         rhs[:], start=True, stop=True)
                    half = (g - glo) % 2
                    if half == 0:
                        gb_t = gop.tile([128, 2, GB, EL], bf16, tag="gb")
                    # cast + t-copy both on the Scalar engine: phase-1's
                    # PE->ACT->store chain must not depend on DVE, whose
                    # in-order queue blocks on phase-2 gather results
                    nc.scalar.activation(gb_t[:, half, :, 0:OUT_C],
                                         ps[:, :, 0:OUT_C], AF.Identity)
                    gf = gb_t[:].bitcast(f32)  # [128, 2, GB, 64]
                    # t (= h@a2) at f32 col 32
                    nc.scalar.activation(gf[:, half, :, 32:33],
                                         ps[:, :, OUT_C:OUT_C + 1],
                                         AF.Identity)
                    # one 1024-row store per pair of groups, alternating
                    # between the two HWDGE queues (sync/scalar)
                    if half == 1 or g == ghi - 1:
                        rowbase = ((g - glo) - half) * 512
                        nrow = (half + 1) * 512
                        eng = nc.sync if (g // 2) % 2 == 0 else nc.scalar
                        eng.dma_start(
                            Gs[s_][rowbase:rowbase + nrow, :].rearrange(
                                "(p j) e -> p j e", p=128),
                            gb_t[:, 0:half + 1, :, :].rearrange(
                                "p a b e -> p (a b) e"))

            _1b_done = set()

            def emit_1b_groups(glo, ghi):
                for g in range(glo, ghi):
                    if g in _1b_done or g >= n_sgroups:
                        continue
                    _1b_done.add(g)
                    xs_t = xin.tile([128, GB * 128], bf16, tag="xs")
                    nc.sync.dma_start(
                        xs_t[:], xS[:, g * GB * 128:(g + 1) * GB * 128])
                    ps2 = psp.tile([128, GB, 1], f32, tag="sps")
                    for j in range(GB):
                        nc.tensor.matmul(
                            ps2[:, j, :], xs_t[:, j * 128:(j + 1) * 128],
                            wa1[:], start=True, stop=True)
                    g0 = g * GB
                    for s_ in secs:
                        lo = max(g0, sec_start[s_])
                        hi = min(g0 + GB, sec_end[s_])
                        if lo < hi:
                            nc.vector.tensor_copy(
                                s_tiles[s_][:, lo - sec_start[s_]:
                                            hi - sec_start[s_]],
                                ps2[:, lo - g0:hi - g0, 0])

            def emit_1b_section(s_):
                if s_ in sec_start:
                    emit_1b_groups(sec_start[s_] // GB,
                                   -(-sec_end[s_] // GB))

            # emission order: p1(s0), p1(s1), 1b(s0), then for s>=2:
            # 1b(s-1), p1(s); finally 1b(last)+stragglers+s_own.
            emit_p1_section(0)
            emit_1b_section(0)
            if NSEC > 1:
                emit_p1_section(1)
                emit_1b_section(1)
            for s_ in range(2, NSEC):
                emit_p1_section(s_)
                emit_1b_section(s_)
            emit_1b_groups(0, n_sgroups)  # any stragglers
            # s_own: s for own nodes in acc-row order, from xT cols [0, NPC).
            # Rotation puts own node q at rotated row q; section 0 has no
            # padding (2560 = SEC_PAD[0]) so xT cols [0, NPC_PAD) are exactly
            # the own nodes (NPC_PAD < SEC_SIZES[0] + SEC_SIZES[1]).
            assert SEC_SIZES[0] == SEC_PAD[0]
            assert NPC_PAD <= SEC_SIZES[0] + SEC_SIZES[1]
            for i3, c0 in enumerate(range(0, nout_cols, GB)):
                cn = min(GB, nout_cols - c0)
                # late nominal time: keeps the DVE copies out of the way of
                # early phase-2 batches (they are only needed by acc2 fill)
                with tc.tile_wait_until(0.32 + 0.002 * i3):
                    xts = xin.tile([128, GB * 128], bf16, tag="xso")
                    nc.sync.dma_start(xts[:, 0:cn * 128],
                                      xT[:, c0 * 128:(c0 + cn) * 128])
                    ps3 = psp.tile([128, GB, 2], f32, tag="sps")
                    for j in range(cn):
                        nc.tensor.matmul(
                            ps3[:, j, :], xts[:, j * 128:(j + 1) * 128],
                            wa12[:], start=True, stop=True)
                    nc.vector.tensor_copy(s_own[:, c0:c0 + cn],
                                          ps3[:, 0:cn, 0])
                    nc.vector.tensor_copy(t_own[:, c0:c0 + cn],
                                          ps3[:, 0:cn, 1])

            # ---------- phase 2 (software-pipelined, skewed emission) ------
            IDX_AHEAD = 3
            git_tiles, sit_tiles = {}, {}

            def emit_idx_load(k):
                if k >= nbat:
                    return
                b = batches[k]
                git = idxp.tile([128, 512], i16, tag="git")
                nc.sync.dma_start(git[:, 0:b["cols"] * 8],
                                  gidx[:, goffs[k]:goffs[k] + b["cols"] * 8])
                git_tiles[k] = git
                sit = idxp.tile([128, 512], i16, tag="sit")
                nc.sync.dma_start(sit[:, 0:b["m"] * 8],
                                  sidx[:, moffs[k]:moffs[k] + b["m"] * 8])
                sit_tiles[k] = sit

            def emit_gather(k):
                b = batches[k]
                cols, sec = b["cols"], b["sec"]
                git = git_tiles.pop(k)
                hg = hgp.tile([128, MAXC, EL], bf16, tag="hg")
                nc.gpsimd.dma_gather(
                    out_ap=hg[:, 0:cols, :],
                    in_ap=Gs[sec][:],
                    idxs_ap=git[:, 0:cols * 8],
                    num_idxs=128 * cols, num_idxs_reg=128 * cols,
                    elem_size=EL, single_packet=False)
                return hg

            def emit_compute_scatter(k, hg):
                b = batches[k]
                cols, mb = b["cols"], b["m"]
                mcols16 = mb * 8
                mo8 = moffs[k] // 8
                sit = sit_tiles.pop(k)
                s_sec = s_tiles[b["sec"]]
                ml = mo8 - sec_start[b["sec"]]
                hgf = hg[:].bitcast(f32)     # [128, MAXC, 64]
                z = ph2.tile([128, MAXC], f32, tag="z")
                co2, mo2 = 0, 0
                for K, m_run in b["runs"]:
                    K = int(K)
                    t4 = hgf[:, co2:co2 + m_run * K, 32:33].rearrange(
                        "p (m k) e -> p m k e", k=K)
                    s4 = s_sec[:, ml + mo2:ml + mo2 + m_run][
                        :, :, None, None]
                    z4 = z[:, co2:co2 + m_run * K].rearrange(
                        "p (m k) -> p m k", k=K)[:, :, :, None]
                    nc.vector.tensor_tensor(
                        out=z4, in0=t4,
                        in1=s4.to_broadcast([128, m_run, K, 1]), op=OP.add)
                    co2 += m_run * K
                    mo2 += m_run
                zl = ph2.tile([128, MAXC], f32, tag="zl")
                w = ph2.tile([128, MAXC], bf16, tag="w")
                prod = ph2.tile([128, MAXC, OUT_C], bf16, tag="prod")
                nc.vector.scalar_tensor_tensor(
                    out=zl[:, 0:cols], in0=z[:, 0:cols], scalar=ALPHA,
                    in1=z[:, 0:cols], op0=OP.mult, op1=OP.max)
                nc.scalar.activation(w[:, 0:cols], zl[:, 0:cols], AF.Exp)
                nc.vector.tensor_tensor(
                    out=prod[:, 0:cols, :], in0=hg[:, 0:cols, 0:OUT_C],
                    in1=w[:, 0:cols, None].to_broadcast([128, cols, OUT_C]),
                    op=OP.mult)
                pf = pfp.tile([128, MAXM, OUT_C + 1], f32, tag="pf")
                co2, mo2 = 0, 0
                for K, m_run in b["runs"]:
                    K = int(K)
                    pv = prod[:, co2:co2 + m_run * K, :].rearrange(
                        "p (m k) e -> p m e k", k=K)
                    nc.vector.tensor_reduce(
                        out=pf[:, mo2:mo2 + m_run, 0:OUT_C], in_=pv,
                        axis=AX.X, op=OP.add)
                    wv = w[:, co2:co2 + m_run * K].rearrange(
                        "p (m k) -> p m k", k=K)
                    nc.vector.tensor_reduce(
                        out=pf[:, mo2:mo2 + m_run, OUT_C:OUT_C + 1],
                        in_=wv, axis=AX.X, op=OP.add)
                    co2 += m_run * K
                    mo2 += m_run
                partial = ph2b.tile([128, MAXM, EL], bf16, tag="partial")
                nc.scalar.activation(partial[:, 0:mb, 0:OUT_C + 1],
                                     pf[:, 0:mb, :], AF.Identity)
                nc.gpsimd.dma_scatter_add(
                    out_ap=(acc0 if k % 2 == 0 else acc1)[:],
                    in_ap=partial[:, 0:mb, :],
                    idxs_ap=sit[:, 0:mcols16],
                    num_idxs=128 * mb, num_idxs_reg=128 * mb,
                    elem_size=EL, single_packet=False, queue_num=1)

            SKEW = 2
            for k in range(IDX_AHEAD):
                emit_idx_load(k)
            hg_tiles = {}
            # nominal 60us/batch timeline: the scheduler's SWDGE cost model
            # is ~20x optimistic, so without this it interleaves phase-2 ops
            # (which stall on gather DMAs) ahead of phase-1 work on the
            # in-order engine queues (head-of-line blocking)
            for k in range(nbat + SKEW):
                with tc.tile_wait_until(0.04 + 0.06 * k):
                    if k < nbat:
                        emit_idx_load(k + IDX_AHEAD)
                        hg_tiles[k] = emit_gather(k)
                    if k >= SKEW:
                        emit_compute_scatter(k - SKEW,
                                             hg_tiles.pop(k - SKEW))

            # ---------- self-term pre-add: acc2 = [w_s*h | w_s] ----------
            # (runs during phase 2; final then just sums three accumulators)
            chunk = 8
            s0c = SEC_SIZES[0] // 128  # own-node cols in section 0 (exact)
            assert SEC_SIZES[0] % 128 == 0
            assert s0c % chunk == 0

            def blocked_pieces(gt, col0, ncols, secpad):
                """Yield (colin_lo, ncols, ap) covering table cols
                [col0, col0+ncols) (col = 128 rows) of a blocked table."""
                done = 0
                while done < ncols:
                    col = col0 + done
                    b = col // 8
                    bs = min(1024, secpad - b * 1024)
                    jd = bs // 128
                    jlo = col - b * 8
                    jn = min(jd - jlo, ncols - done)
                    ap = gt[b * 1024:b * 1024 + bs, :].rearrange(
                        "(p j) e -> p j e", p=128)[:, jlo:jlo + jn, :]
                    yield done, jn, ap
                    done += jn

            for i4, c0 in enumerate(range(0, nout_cols, chunk)):
                cn = min(chunk, nout_cols - c0)
                with tc.tile_wait_until(0.40 + 0.05 * i4):
                    atg = fin2.tile([128, chunk, EL], bf16, tag="atg")
                    gsel = ((Gs[0], c0, min(cn, s0c - c0), SEC_PAD[0], 0)
                            if c0 < s0c else
                            (Gs[1], c0 - s0c, cn, SEC_PAD[1], 0))
                    gt, gcol0, gcn, gpad, dc0 = gsel
                    for dcol, jn, ap in blocked_pieces(gt, gcol0, gcn, gpad):
                        nc.sync.dma_start(
                            atg[:, dc0 + dcol:dc0 + dcol + jn, :], ap)
                    zs = fin2.tile([128, chunk], f32, tag="zs")
                    nc.vector.tensor_tensor(
                        out=zs[:, 0:cn], in0=s_own[:, c0:c0 + cn],
                        in1=t_own[:, c0:c0 + cn], op=OP.add)
                    nc.vector.scalar_tensor_tensor(
                        out=zs[:, 0:cn], in0=zs[:, 0:cn], scalar=ALPHA,
                        in1=zs[:, 0:cn], op0=OP.mult, op1=OP.max)
                    ws = fin2.tile([128, chunk], f32, tag="ws")
                    nc.scalar.activation(ws[:, 0:cn], zs[:, 0:cn], AF.Exp)
                    pr = fin2.tile([128, chunk, EL], bf16, tag="pr")
                    nc.vector.tensor_tensor(
                        out=pr[:, 0:cn, 0:OUT_C], in0=atg[:, 0:cn, 0:OUT_C],
                        in1=ws[:, 0:cn, None].to_broadcast(
                            [128, cn, OUT_C]), op=OP.mult)
                    nc.vector.tensor_copy(pr[:, 0:cn, OUT_C:OUT_C + 1],
                                          ws[:, 0:cn, None])
                    bs2 = min(1024, NPC_PAD - (c0 // 8) * 1024)
                    nc.scalar.dma_start(
                        acc2[(c0 // 8) * 1024:(c0 // 8) * 1024 + bs2, :]
                        .rearrange("(p j) e -> p j e", p=128),
                        pr[:, 0:cn, :])

            # ---------- final: out = (acc0+acc1+acc2.h)/(e0+e1+e2) ----------
            for c0 in range(0, nout_cols, chunk):
                cn = min(chunk, nout_cols - c0)
                at0 = fin.tile([128, chunk, EL], bf16, tag="at0")
                at1 = fin.tile([128, chunk, EL], bf16, tag="at1")
                at2 = fin.tile([128, chunk, EL], bf16, tag="at2")
                for at, accd, pad in ((at0, acc0, NPC_PAD),
                                      (at1, acc1, NPC_PAD),
                                      (at2, acc2, NPC_PAD)):
                    for dcol, jn, ap in blocked_pieces(accd, c0, cn, pad):
                        nc.sync.dma_start(at[:, dcol:dcol + jn, :], ap)
                esum = fin.tile([128, chunk], f32, tag="esum")
                nc.vector.tensor_tensor(
                    out=esum[:, 0:cn, None],
                    in0=at0[:, 0:cn, OUT_C:OUT_C + 1],
                    in1=at1[:, 0:cn, OUT_C:OUT_C + 1], op=OP.add)
                nc.vector.tensor_tensor(
                    out=esum[:, 0:cn, None], in0=esum[:, 0:cn, None],
                    in1=at2[:, 0:cn, OUT_C:OUT_C + 1], op=OP.add)
                rec = fin.tile([128, chunk], f32, tag="rec")
                nc.vector.reciprocal(rec[:, 0:cn], esum[:, 0:cn])
                hsum = fin.tile([128, chunk, OUT_C], f32, tag="hsum")
                nc.vector.tensor_tensor(
                    out=hsum[:, 0:cn, :], in0=at0[:, 0:cn, 0:OUT_C],
                    in1=at1[:, 0:cn, 0:OUT_C], op=OP.add)
                nc.vector.tensor_tensor(
                    out=hsum[:, 0:cn, :], in0=hsum[:, 0:cn, :],
                    in1=at2[:, 0:cn, 0:OUT_C], op=OP.add)
                ot = fin.tile([128, chunk, OUT_C], f32, tag="ot")
                nc.vector.tensor_tensor(
                    out=ot[:, 0:cn, :], in0=hsum[:, 0:cn, :],
                    in1=rec[:, 0:cn, None].to_broadcast([128, cn, OUT_C]),
                    op=OP.mult)
                bs2 = min(1024, NPC_PAD - (c0 // 8) * 1024)
                nc.scalar.dma_start(
                    out[(c0 // 8) * 1024:(c0 // 8) * 1024 + bs2, :].rearrange(
                        "(p j) e -> p j e", p=128), ot[:, 0:cn, :])
    nc.compile()
    return nc


_CACHE = {}
_LAST = {}  # debug/timing introspection: nc + in_maps of last call


def kernel(x, W, a, edge_index):
    import ml_dtypes
    from concourse.bass_utils import run_bass_kernel_spmd

    bf16_t = ml_dtypes.bfloat16

    x = np.asarray(x, np.float32)
    W = np.asarray(W, np.float32)
    a = np.asarray(a, np.float32)
    meta, per_core = _prep(edge_index)

    key = (N, E, tuple((b["sec"], tuple(b["runs"])) for b in meta["batches"]))
    if key not in _CACHE:
        _CACHE[key] = _build_program(meta)
    nc = _CACHE[key]

    xTf = np.ascontiguousarray(x.T)  # [128, N]
    aTv = np.ascontiguousarray(a.reshape(2, OUT_C).T)  # [64, 2]
    mcols_total = meta["mcols"]
    mcols_pad = -(-mcols_total // GB) * GB
    in_maps = []
    for c in range(NCORES):
        n0 = c * NPC
        xrot_full = np.concatenate([xTf[:, n0:], xTf[:, :n0]], axis=1)
        # per-section column blocks, each zero-padded to SEC_PAD[s]
        blocks = []
        for s_ in range(NSEC):
            blk = xrot_full[:, SEC_OFF[s_]:SEC_OFF[s_ + 1]]
            pad = SEC_PAD[s_] - SEC_SIZES[s_]
            if pad:
                blk = np.concatenate(
                    [blk, np.zeros((128, pad), np.float32)], axis=1)
            blocks.append(blk)
        xrot = np.ascontiguousarray(
            np.concatenate(blocks, axis=1).astype(bf16_t))
        ids = per_core[c]["sorder"].T.reshape(-1)  # q = j*128+p
        xs = np.zeros((mcols_pad * 128, IN_C), np.float32)
        valid = ids < NPC
        xs[:len(ids)][valid] = x[n0 + ids[valid]]
        in_maps.append({
            "xT": xrot, "W": W, "aT": aTv,
            "xS": np.ascontiguousarray(xs.T.astype(bf16_t)),
            "gidx": per_core[c]["gidx"],
            "sidx": per_core[c]["sidx"],
        })
    _LAST["nc"] = nc
    _LAST["in_maps"] = in_maps
    res = run_bass_kernel_spmd(nc, in_maps, core_ids=list(range(NCORES)))
    qmap = _g_maps()[NSEC][:NPC]
    outs = [res.results[c]["out"][qmap] for c in range(NCORES)]
    return np.concatenate(outs, axis=0)
